# revision 5
# baseline (speedup 1.0000x reference)
"""Self-contained Trainium2 kernel for nn_Encoder (DA-RNN input-attention LSTM).

reference:
    Ud = einsum('btn,ut->bnu', data, U_e)                    # [B, n, T]
    per step t:
        q = [h; c]                                           # [B, 2m]
        Wq = q @ W_e.T                                       # [B, T]
        e = tanh(Wq[:,None,:] + Ud) @ v                      # [B, n]
        alpha = softmax(e, -1); x_til = x_t * alpha
        LSTM cell (pytorch gate order i,f,g,o) -> h, c
    returns H [B, T, m], alphas [B, T, n]

Sharding: data-parallel over batch B=256 across 8 cores (32 rows each);
all weights replicated; no cross-core communication.

Per-core layouts (P = 128 partitions):
    big attention tensor: [u_lo(128 part), (u_hi 2, b 32, n 128) free]
    state kept both b-major ([32, 256]) and transposed ([m(2x128 part), b])
"""

import numpy as np

import concourse.bass as bass
import concourse.tile as tile
from concourse import mybir
from concourse.bass_utils import run_bass_kernel_spmd

AF = mybir.ActivationFunctionType
ALU = mybir.AluOpType
AX = mybir.AxisListType
F32 = mybir.dt.float32
BF16 = mybir.dt.bfloat16

B_FULL, T, N, M = 256, 256, 128, 256
NCORES = 8
B = B_FULL // NCORES          # 32 per core
TWO_M = 2 * M                 # 512
G4 = 4 * M                    # 1024 (gates, reordered i,f,o,g)
UHI = 2                       # T = 2*128 attention-index tiles

ATT_BF16 = False              # attention-path dtype knob
T_STEPS = T

_wsplit_ctr = [0]


def split_sync_waits(nc, max_waits=1):
    """This container's walrus accepts only ONE sem wait per instruction.
    Move excess waits onto same-engine NoOps inserted before the instruction."""
    n_split = 0
    for f in nc.m.functions:
        for blk in f.blocks:
            new = []
            for ins in blk.instructions:
                si = ins.sync_info
                if si is not None and len(si.on_wait) > max_waits:
                    waits = list(si.on_wait)
                    for w in waits[max_waits:]:
                        _wsplit_ctr[0] += 1
                        nop = mybir.InstNoOp(
                            name=f"WSPLIT-{_wsplit_ctr[0]}", ins=[], outs=[])
                        nop.engine = ins.engine
                        nop.sync_info = mybir.SyncInfo(on_wait=[w], on_update=[])
                        new.append(nop)
                    ins.sync_info = mybir.SyncInfo(
                        on_wait=waits[:max_waits], on_update=list(si.on_update))
                    n_split += 1
                new.append(ins)
            blk.instructions = new
    return n_split


def _bcast(ap, n):
    """Append a stride-0 broadcast dim of size n to an AP."""
    return bass.AP(ap.tensor, ap.offset, list(ap.ap) + [[0, n]])


def build_nc(t_steps=T_STEPS, att_bf16=ATT_BF16):
    ATT = BF16 if att_bf16 else F32
    nc = bass.Bass()

    d_dtn = nc.dram_tensor("dataTn", [N, T * B], F32, kind="ExternalInput")
    d_datt = nc.dram_tensor("data_att", [B, T, N], ATT, kind="ExternalInput")
    d_h0T = nc.dram_tensor("h0T", [M, B], F32, kind="ExternalInput")
    d_c0T = nc.dram_tensor("c0T", [M, B], F32, kind="ExternalInput")
    d_c0b = nc.dram_tensor("c0b", [B, M], F32, kind="ExternalInput")
    d_weT = nc.dram_tensor("W_eT", [TWO_M, T], F32, kind="ExternalInput")
    d_ueT = nc.dram_tensor("U_eT", [T, T], ATT, kind="ExternalInput")
    d_vd = nc.dram_tensor("v_delta", [128, UHI * B * B], ATT, kind="ExternalInput")
    d_wih = nc.dram_tensor("W_ihT", [N, G4], F32, kind="ExternalInput")
    d_whh = nc.dram_tensor("W_hhT", [M, G4], F32, kind="ExternalInput")
    d_bias = nc.dram_tensor("bias", [1, G4], F32, kind="ExternalInput")
    d_i32 = nc.dram_tensor("I32", [B, B], F32, kind="ExternalInput")

    d_H = nc.dram_tensor("H", [B, T, M], F32, kind="ExternalOutput")
    d_A = nc.dram_tensor("attn", [B, T, N], F32, kind="ExternalOutput")

    with tile.TileContext(nc) as tc:
        with tc.tile_pool(name="singles", bufs=1) as sp:
            ud_sb = sp.tile([128, UHI * B * N], ATT)     # (uh, b, n)
            th_sb = sp.tile([128, UHI * B * N], ATT)     # tanh workspace (in-place)
            dtn_sb = sp.tile([N, T * B], F32)            # (t, b)
            weT_sb = sp.tile([128, 4 * T], F32)          # (kt 4, u 256)
            ueT_sb = sp.tile([128, 2 * T], ATT)          # (th 2, u 256)
            vd_sb = sp.tile([128, UHI * B * B], ATT)     # (uh, b, j)
            wih_sb = sp.tile([N, G4], F32)
            whh_sb = sp.tile([128, 2 * G4], F32)         # (kh 2, g 1024)
            bias_sb = sp.tile([1, G4], F32)
            i32_sb = sp.tile([B, B], F32)
            ones_sb = sp.tile([1, B], F32)
            hT_sb = sp.tile([128, 2 * B], F32)           # (mh, b)
            cT_sb = sp.tile([128, 2 * B], F32)
            cb_sb = sp.tile([B, M], F32)
            wq_sb = sp.tile([128, UHI * B], ATT)         # (uh, b)

            # ---------------- init: loads ----------------
            nc.sync.dma_start(dtn_sb[:], d_dtn[:])
            for kt in range(4):
                nc.sync.dma_start(weT_sb[:, kt * T:(kt + 1) * T],
                                  d_weT[kt * 128:(kt + 1) * 128, :])
            for th in range(2):
                nc.sync.dma_start(ueT_sb[:, th * T:(th + 1) * T],
                                  d_ueT[th * 128:(th + 1) * 128, :])
            nc.sync.dma_start(vd_sb[:], d_vd[:])
            nc.sync.dma_start(wih_sb[:], d_wih[:])
            for kh in range(2):
                nc.sync.dma_start(whh_sb[:, kh * G4:(kh + 1) * G4],
                                  d_whh[kh * 128:(kh + 1) * 128, :])
            nc.sync.dma_start(bias_sb[:], d_bias[:])
            nc.sync.dma_start(i32_sb[:], d_i32[:])
            nc.sync.dma_start(cb_sb[:], d_c0b[:])
            # h0T/c0T [256, 32] -> [(mh ml), b] -> [ml, (mh b)]
            nc.sync.dma_start(
                hT_sb[:].rearrange("p (mh b) -> p mh b", mh=2),
                d_h0T[:].rearrange("(mh p) b -> p mh b", mh=2))
            nc.sync.dma_start(
                cT_sb[:].rearrange("p (mh b) -> p mh b", mh=2),
                d_c0T[:].rearrange("(mh p) b -> p mh b", mh=2))
            nc.vector.memset(ones_sb[:], 1.0)

            # ---------------- init: Ud = einsum('btn,ut->bnu') ----------------
            with tc.tile_pool(name="ud_init", bufs=4) as ip, \
                 tc.tile_pool(name="ud_ps", bufs=2, space="PSUM") as ipp:
                for b in range(B):
                    dt0 = ip.tile([128, N], ATT, tag="dt")
                    dt1 = ip.tile([128, N], ATT, tag="dt")
                    nc.sync.dma_start(dt0[:], d_datt[b, 0:128, :])
                    nc.sync.dma_start(dt1[:], d_datt[b, 128:256, :])
                    for uh in range(UHI):
                        ups = ipp.tile([128, N], F32, tag="ups")
                        nc.tensor.matmul(
                            ups[:], ueT_sb[:, uh * 128:(uh + 1) * 128],
                            dt0[:], start=True, stop=False)
                        nc.tensor.matmul(
                            ups[:], ueT_sb[:, T + uh * 128:T + (uh + 1) * 128],
                            dt1[:], start=False, stop=True)
                        nc.scalar.copy(
                            ud_sb[:, (uh * B + b) * N:(uh * B + b + 1) * N],
                            ups[:])

            # ---------------- step loop ----------------
            with tc.tile_pool(name="wk", bufs=2) as wk, \
                 tc.tile_pool(name="ps_wq", bufs=1, space="PSUM") as ps_wq, \
                 tc.tile_pool(name="ps_e", bufs=1, space="PSUM") as ps_e, \
                 tc.tile_pool(name="ps_g", bufs=1, space="PSUM") as ps_g, \
                 tc.tile_pool(name="ps_tr", bufs=1, space="PSUM") as ps_tr:
                for t in range(t_steps):
                    # 1) Wq^T [u, b] = W_e @ [h; c]  (f32 matmuls)
                    wq_ps = ps_wq.tile([128, UHI * B], F32, tag="wq")
                    q_tiles = [hT_sb[:, 0:B], hT_sb[:, B:2 * B],
                               cT_sb[:, 0:B], cT_sb[:, B:2 * B]]
                    for uh in range(UHI):
                        for kt in range(4):
                            nc.tensor.matmul(
                                wq_ps[:, uh * B:(uh + 1) * B],
                                weT_sb[:, kt * T + uh * 128:kt * T + (uh + 1) * 128],
                                q_tiles[kt],
                                start=(kt == 0), stop=(kt == 3))
                    nc.vector.tensor_copy(wq_sb[:], wq_ps[:])

                    # 2-3) tanh(Ud + Wq) in two u_hi chunks; V add, S tanh in-place
                    for uh in range(UHI):
                        sl = slice(uh * B * N, (uh + 1) * B * N)
                        dst = th_sb[:, sl].rearrange("p (b n) -> p b n", b=B)
                        src = ud_sb[:, sl].rearrange("p (b n) -> p b n", b=B)
                        wqv = _bcast(wq_sb[:, uh * B:(uh + 1) * B], N)
                        nc.vector.tensor_tensor(dst, src, wqv, op=ALU.add)
                        nc.scalar.activation(th_sb[:, sl], th_sb[:, sl], AF.Tanh)

                    # 4) e[b, n] = sum_u v[u] tanh[...]  (delta-masked weights)
                    e_ps = ps_e.tile([B, N], F32, tag="e")
                    for uh in range(UHI):
                        for b in range(B):
                            nc.tensor.matmul(
                                e_ps[:],
                                vd_sb[:, (uh * B + b) * B:(uh * B + b + 1) * B],
                                th_sb[:, (uh * B + b) * N:(uh * B + b + 1) * N],
                                start=(uh == 0 and b == 0),
                                stop=(uh == 1 and b == B - 1))

                    # 5) softmax over n
                    negmax = wk.tile([B, 1], F32, tag="negmax")
                    nc.vector.tensor_reduce(negmax[:], e_ps[:], axis=AX.X,
                                            op=ALU.max, negate=True)
                    p_sb = wk.tile([B, N], F32, tag="p")
                    esum = wk.tile([B, 1], F32, tag="esum")
                    nc.scalar.activation(p_sb[:], e_ps[:], AF.Exp,
                                         bias=negmax[:], accum_out=esum[:])
                    rsum = wk.tile([B, 1], F32, tag="rsum")
                    nc.vector.reciprocal(rsum[:], esum[:])
                    alpha = wk.tile([B, N], F32, tag="alpha")
                    nc.vector.tensor_scalar_mul(alpha[:], p_sb[:], rsum[:])
                    nc.sync.dma_start(d_A[:, t, :], alpha[:])

                    # 6) x_til^T [n, b] = alpha^T * x_t^T
                    aT_ps = ps_tr.tile([N, B], F32, tag="aT")
                    nc.tensor.transpose(aT_ps[:], alpha[:], i32_sb[:])
                    xtT = wk.tile([N, B], F32, tag="xtT")
                    nc.vector.tensor_tensor(
                        xtT[:], aT_ps[:], dtn_sb[:, t * B:(t + 1) * B],
                        op=ALU.mult)

                    # 7) gates [b, g] (order i,f,o,g), f32
                    g_ps = [ps_g.tile([B, 512], F32, tag=f"g{c}", name=f"g_ps{c}")
                            for c in range(2)]
                    for c in range(2):
                        sl = slice(c * 512, (c + 1) * 512)
                        nc.tensor.matmul(g_ps[c][:], xtT[:], wih_sb[:, sl],
                                         start=True, stop=False)
                        nc.tensor.matmul(g_ps[c][:], hT_sb[:, 0:B],
                                         whh_sb[:, c * 512:(c + 1) * 512],
                                         start=False, stop=False)
                        nc.tensor.matmul(g_ps[c][:], hT_sb[:, B:2 * B],
                                         whh_sb[:, G4 + c * 512:G4 + (c + 1) * 512],
                                         start=False, stop=False)
                        nc.tensor.matmul(g_ps[c][:], ones_sb[:], bias_sb[:, sl],
                                         start=False, stop=True)

                    # 8) LSTM cell via tanh only (sigmoid = .5 + .5*tanh(x/2))
                    tif = wk.tile([B, 512], F32, tag="tif")      # ti | tf
                    nc.scalar.activation(tif[:], g_ps[0][:], AF.Tanh, scale=0.5)
                    tto = wk.tile([B, M], F32, tag="tto")
                    nc.scalar.activation(tto[:], g_ps[1][:, 0:M], AF.Tanh, scale=0.5)
                    tg = wk.tile([B, M], F32, tag="tg")
                    nc.scalar.activation(tg[:], g_ps[1][:, M:2 * M], AF.Tanh)

                    p1 = wk.tile([B, M], F32, tag="p1")          # (1+tf) * c
                    nc.vector.scalar_tensor_tensor(p1[:], tif[:, M:2 * M], 1.0,
                                                   cb_sb[:], op0=ALU.add,
                                                   op1=ALU.mult)
                    p2 = wk.tile([B, M], F32, tag="p2")          # (1+ti) * tg
                    nc.vector.scalar_tensor_tensor(p2[:], tif[:, 0:M], 1.0,
                                                   tg[:], op0=ALU.add,
                                                   op1=ALU.mult)
                    c4 = wk.tile([B, M], F32, tag="c4")          # 2*c_new
                    nc.vector.tensor_tensor(c4[:], p1[:], p2[:], op=ALU.add)
                    nc.vector.tensor_scalar_mul(cb_sb[:], c4[:], 0.5)
                    tch = wk.tile([B, M], F32, tag="tch")        # tanh(c_new)
                    nc.scalar.activation(tch[:], c4[:], AF.Tanh, scale=0.5)
                    h2 = wk.tile([B, M], F32, tag="h2")          # (1+to)*tanh(c)
                    nc.vector.scalar_tensor_tensor(h2[:], tto[:], 1.0, tch[:],
                                                   op0=ALU.add, op1=ALU.mult)
                    hb = wk.tile([B, M], F32, tag="hb")
                    nc.vector.tensor_scalar_mul(hb[:], h2[:], 0.5)
                    nc.sync.dma_start(d_H[:, t, :], hb[:])

                    # 9) refresh transposed state for next step
                    for mh in range(2):
                        trh = ps_tr.tile([128, B], F32, tag="trh")
                        nc.tensor.transpose(trh[:], hb[:, mh * 128:(mh + 1) * 128],
                                            i32_sb[:])
                        nc.scalar.copy(hT_sb[:, mh * B:(mh + 1) * B], trh[:])
                        trc = ps_tr.tile([128, B], F32, tag="trc")
                        nc.tensor.transpose(trc[:], cb_sb[:, mh * 128:(mh + 1) * 128],
                                            i32_sb[:])
                        nc.vector.tensor_copy(cT_sb[:, mh * B:(mh + 1) * B], trc[:])

    split_sync_waits(nc)
    return nc


def _prep_core_inputs(inputs, att_bf16=ATT_BF16):
    """Full inputs -> list of per-core input dicts (host-side layout prep)."""
    att_np = np.dtype("bfloat16") if att_bf16 else np.float32
    data = np.ascontiguousarray(np.asarray(inputs["data"], dtype=np.float32))
    h0 = np.asarray(inputs["h_0"], dtype=np.float32)[0]      # [B_FULL, M]
    c0 = np.asarray(inputs["s_0"], dtype=np.float32)[0]
    W_e = np.asarray(inputs["W_e"], dtype=np.float32)        # [T, 2M]
    U_e = np.asarray(inputs["U_e"], dtype=np.float32)        # [T, T]
    v_e = np.asarray(inputs["v_e"], dtype=np.float32)[0]     # [T]
    W_ih = np.asarray(inputs["W_ih"], dtype=np.float32)      # [4M, N]
    W_hh = np.asarray(inputs["W_hh"], dtype=np.float32)      # [4M, M]
    b_sum = (np.asarray(inputs["b_ih"], dtype=np.float32)
             + np.asarray(inputs["b_hh"], dtype=np.float32))  # [4M]

    # reorder gates (i, f, g, o) -> (i, f, o, g)
    perm = np.concatenate([np.arange(0, M), np.arange(M, 2 * M),
                           np.arange(3 * M, 4 * M), np.arange(2 * M, 3 * M)])
    W_ihT = np.ascontiguousarray(W_ih[perm].T)               # [N, 4M]
    W_hhT = np.ascontiguousarray(W_hh[perm].T)               # [M, 4M]
    bias = np.ascontiguousarray(b_sum[perm])[None, :]        # [1, 4M]

    W_eT = np.ascontiguousarray(W_e.T)                       # [2M, T]
    U_eT = np.ascontiguousarray(U_e.T).astype(att_np)        # [T, T]
    # v_delta[u_lo, (uh, b, j)] = v[uh*128+u_lo] * (j==b)
    v_cols = v_e.reshape(UHI, 128).T                         # [128, 2]
    v_delta = np.einsum("pu,bj->pubj", v_cols,
                        np.eye(B, dtype=np.float32))
    v_delta = np.ascontiguousarray(
        v_delta.reshape(128, UHI * B * B)).astype(att_np)
    i32 = np.eye(B, dtype=np.float32)

    shared = {
        "W_eT": W_eT, "U_eT": U_eT, "v_delta": v_delta,
        "W_ihT": W_ihT, "W_hhT": W_hhT, "bias": bias, "I32": i32,
    }
    in_maps = []
    for c in range(NCORES):
        sl = slice(c * B, (c + 1) * B)
        dl = data[sl]                                        # [32, T, N]
        m = dict(shared)
        m["dataTn"] = np.ascontiguousarray(
            dl.transpose(2, 1, 0).reshape(N, T * B))
        m["data_att"] = np.ascontiguousarray(dl).astype(att_np)
        m["h0T"] = np.ascontiguousarray(h0[sl].T)
        m["c0T"] = np.ascontiguousarray(c0[sl].T)
        m["c0b"] = np.ascontiguousarray(c0[sl])
        in_maps.append(m)
    return in_maps


_NC_CACHE = {}
_RUNNER_CACHE = {}


def _get_nc(t_steps=T_STEPS, att_bf16=ATT_BF16):
    key = (t_steps, att_bf16)
    if key not in _NC_CACHE:
        _NC_CACHE[key] = build_nc(t_steps, att_bf16)
    return _NC_CACHE[key]


def _make_runner(nc):
    """Cached jitted shard_map executor over 8 cores (mirrors
    bass2jax.run_bass_via_pjrt but reusable across calls for benching)."""
    import jax
    from jax.experimental.shard_map import shard_map
    from jax.sharding import Mesh, PartitionSpec
    from concourse.bass2jax import (_bass_exec_p, partition_id_tensor,
                                    install_neuronx_cc_hook)
    install_neuronx_cc_hook()

    partition_name = (nc.partition_id_tensor.name
                      if nc.partition_id_tensor else None)
    in_names, out_names, out_avals = [], [], []
    for alloc in nc.m.functions[0].allocations:
        if not isinstance(alloc, mybir.MemoryLocationSet):
            continue
        name = alloc.memorylocations[0].name
        if alloc.kind == "ExternalInput":
            if name != partition_name:
                in_names.append(name)
        elif alloc.kind == "ExternalOutput":
            out_names.append(name)
            out_avals.append(jax.core.ShapedArray(
                tuple(alloc.tensor_shape), mybir.dt.np(alloc.dtype)))
    n_params = len(in_names)
    all_in = list(in_names) + list(out_names)
    if partition_name is not None:
        all_in.append(partition_name)
    donate = tuple(range(n_params, n_params + len(out_names)))

    def _body(*args):
        operands = list(args)
        if partition_name is not None:
            operands.append(partition_id_tensor())
        outs = _bass_exec_p.bind(
            *operands,
            out_avals=tuple(out_avals),
            in_names=tuple(all_in),
            out_names=tuple(out_names),
            lowering_input_output_aliases=(),
            sim_require_finite=True,
            sim_require_nnan=True,
            nc=nc,
        )
        return tuple(outs)

    mesh = Mesh(np.asarray(jax.devices()[:NCORES]), ("core",))
    in_specs = (PartitionSpec("core"),) * (n_params + len(out_names))
    out_specs = (PartitionSpec("core"),) * len(out_names)
    sharded = jax.jit(
        shard_map(_body, mesh=mesh, in_specs=in_specs,
                  out_specs=out_specs, check_rep=False),
        donate_argnums=donate, keep_unused=True)
    return {"fn": sharded, "in_names": in_names, "out_names": out_names,
            "out_avals": out_avals, "mesh": mesh}


def _get_runner(t_steps=T_STEPS, att_bf16=ATT_BF16):
    key = (t_steps, att_bf16)
    if key not in _RUNNER_CACHE:
        _RUNNER_CACHE[key] = _make_runner(_get_nc(t_steps, att_bf16))
    return _RUNNER_CACHE[key]


def _device_inputs(runner, in_maps):
    import jax
    from jax.sharding import NamedSharding, PartitionSpec
    sh = NamedSharding(runner["mesh"], PartitionSpec("core"))
    return [jax.device_put(
        np.concatenate([np.asarray(m[k]) for m in in_maps], axis=0), sh)
        for k in runner["in_names"]]


def _fresh_zeros(runner):
    import jax
    from jax.sharding import NamedSharding, PartitionSpec
    sh = NamedSharding(runner["mesh"], PartitionSpec("core"))
    return [jax.device_put(
        np.zeros((NCORES * a.shape[0], *a.shape[1:]), a.dtype), sh)
        for a in runner["out_avals"]]


def _run_once(runner, dev_in):
    import jax
    outs = runner["fn"](*dev_in, *_fresh_zeros(runner))
    jax.block_until_ready(outs)
    return {n: np.asarray(o) for n, o in zip(runner["out_names"], outs)}


def kernel(**inputs):
    runner = _get_runner()
    in_maps = _prep_core_inputs(inputs)
    out = _run_once(runner, _device_inputs(runner, in_maps))
    H = out["H"].reshape(NCORES * B, T, M)
    A = out["attn"].reshape(NCORES * B, T, N)
    return H.astype(np.float32), A.astype(np.float32)


def bench(inputs, iters=10):
    """Returns (outputs_dict, per-call wall times in seconds)."""
    import time as _time
    import jax
    runner = _get_runner()
    in_maps = _prep_core_inputs(inputs)
    dev_in = _device_inputs(runner, in_maps)
    out = _run_once(runner, dev_in)            # warmup + compile
    zs = [_fresh_zeros(runner) for _ in range(iters)]
    times = []
    for z in zs:
        t0 = _time.perf_counter()
        o = runner["fn"](*dev_in, *z)
        jax.block_until_ready(o)
        times.append(_time.perf_counter() - t0)
    return out, times
